# revision 17
# baseline (speedup 1.0000x reference)
"""Trainium2 Bass kernel for nn_AttributeDecoder (gather + per-head small linear).

  logits[k, s, v] = features.reshape(-1, 256)[mask_idx[k, s], :] @ W[k] + b[k]
  K=24 heads, S=16384 positions/head, D=256, V=8, N=131072 table rows.

Sharding: expert-parallel over heads — 3 heads per core x 8 cores, features
table replicated (per-core DRAM copy, gathered via dma_gather).

Per (core, head): indices bucketed by table region (so in-bucket offsets fit
dma_gather's int16 indices), padded to a fixed bucket size; gathered rows are
processed in <=512-position blocks:
  fp32 mode  : PE transpose -> fp32 matmul (exact)
  bf16x2 mode: split bf16 hi/lo table gathered pre-transposed (xbar) ->
               4 bf16 matmuls accumulated in PSUM (abs err ~2e-5)
Host unpermutes the bucketed output order.
"""
import os
import numpy as np

import concourse.bass as bass
import concourse.mybir as mybir
import concourse.tile as tile
from concourse import bacc
from concourse.bass_utils import run_bass_kernel_spmd
from concourse.masks import make_identity

NCORES = 8
KH = 3                 # heads per core
P = 128
D = 256
V = 8
S = 16384
NROWS = 131072

MODE = os.environ.get("KERNEL_MODE", "fp32")   # "fp32" (exact) | "bf16x2"

# per-mode gather geometry
GEO = {
    # NBUCK buckets of NROWS/NBUCK rows; BS padded positions per bucket;
    # BLOCKS: matmul n-block widths covering BS positions
    "fp32":   dict(NBUCK=4, BS=4608, BLOCKS=[512] * 9),
    "bf16x2": dict(NBUCK=8, BS=2304, BLOCKS=[512] * 4 + [256]),
}
SW = 18432             # NBUCK * BS for both modes (device positions per head)
assert all(g["NBUCK"] * g["BS"] == SW and sum(g["BLOCKS"]) == g["BS"]
           for g in GEO.values())

f32 = mybir.dt.float32
bf16 = mybir.dt.bfloat16
i16 = mybir.dt.int16

_NC_CACHE = {}


def build_nc(mode=MODE, loop_k=None):
    NBUCK = GEO[mode]["NBUCK"]
    BS = GEO[mode]["BS"]
    BLOCKS = GEO[mode]["BLOCKS"]
    BROWS = NROWS // NBUCK
    HB = KH * NBUCK
    TILES = BS // P

    nc = bacc.Bacc("TRN2", target_bir_lowering=False, debug=False,
                   num_swdge_queues=4)
    if mode == "fp32":
        feat = nc.dram_tensor("feat", [NROWS, D], f32, kind="ExternalInput")
        w = nc.dram_tensor("w", [P, KH * 2 * V], f32, kind="ExternalInput")
    else:
        feat = nc.dram_tensor("feat", [NROWS, 2 * D], bf16, kind="ExternalInput")
        w = nc.dram_tensor("w", [P, KH * 2 * 2 * V], bf16, kind="ExternalInput")
    idx = nc.dram_tensor("idx", [HB, P, BS // 16], i16, kind="ExternalInput")
    bias = nc.dram_tensor("bias", [V, KH], f32, kind="ExternalInput")
    out = nc.dram_tensor("out", [KH, V, SW], f32, kind="ExternalOutput")

    with tile.TileContext(nc) as tc:
        with tc.tile_pool(name="const", bufs=1) as cpool, \
             tc.tile_pool(name="gath", bufs=4) as gpool, \
             tc.tile_pool(name="gt", bufs=4) as gtpool, \
             tc.tile_pool(name="ob", bufs=2) as obpool, \
             tc.tile_pool(name="pst", bufs=3, space="PSUM") as ptpool, \
             tc.tile_pool(name="pso", bufs=2, space="PSUM") as popool:

            w_sb = cpool.tile(list(w.shape), w.dtype)
            nc.sync.dma_start(w_sb[:], w[:])
            bias_sb = cpool.tile([V, KH], f32)
            nc.sync.dma_start(bias_sb[:], bias[:])
            idx_sb = cpool.tile([P, HB, BS // 16], i16)
            for hb in range(HB):
                nc.sync.dma_start(idx_sb[:, hb, :], idx[hb])
            ident = None
            if mode == "fp32":
                ident = cpool.tile([P, P], f32)
                make_identity(nc, ident[:])

            import contextlib
            loop_cm = tc.For_i(0, loop_k, 1) if loop_k else contextlib.nullcontext()
            with loop_cm:
                for h in range(KH):
                    for b in range(NBUCK):
                        hb = h * NBUCK + b
                        if mode == "fp32":
                            g = gpool.tile([P, TILES, D], f32, tag="g")
                            # split across all 4 SWDGE queues for more
                            # outstanding HBM reads
                            HT = TILES // 4
                            HN = BS // 4
                            for q in range(4):
                                nc.gpsimd.dma_gather(
                                    g[:, q * HT:(q + 1) * HT, :],
                                    feat[b * BROWS:(b + 1) * BROWS, :],
                                    idx_sb[:, hb, q * (HN // 16):(q + 1) * (HN // 16)],
                                    HN, HN, D,
                                    single_packet=False, queue_num=q)
                        else:
                            g = gpool.tile([P, 4, BS], bf16, tag="g")
                            nc.gpsimd.dma_gather(
                                g[:], feat[b * BROWS:(b + 1) * BROWS, :],
                                idx_sb[:, hb, :], BS, BS, 2 * D,
                                transpose=True, single_packet=False)
                        off = 0
                        for blk, NW in enumerate(BLOCKS):
                            if mode == "fp32":
                                pt0 = ptpool.tile([P, 512], f32, tag="pt0")
                                pt1 = ptpool.tile([P, 512], f32, tag="pt1")
                                for tl in range(NW // P):
                                    t = off // P + tl
                                    nc.tensor.transpose(
                                        out=pt0[:, tl * P:(tl + 1) * P],
                                        in_=g[:, t, 0:P], identity=ident[:])
                                    nc.tensor.transpose(
                                        out=pt1[:, tl * P:(tl + 1) * P],
                                        in_=g[:, t, P:D], identity=ident[:])
                                gt0 = gtpool.tile([P, 512], f32, tag="gt0")
                                gt1 = gtpool.tile([P, 512], f32, tag="gt1")
                                nc.vector.tensor_copy(gt0[:, :NW], pt0[:, :NW])
                                # second copy on the idle scalar engine to
                                # take it off the DVE track
                                nc.scalar.activation(
                                    gt1[:, :NW], pt1[:, :NW],
                                    mybir.ActivationFunctionType.Identity,
                                    bias=0.0)
                                po = popool.tile([V, 512], f32, tag="po")
                                nc.tensor.matmul(
                                    po[:, :NW],
                                    lhsT=w_sb[:, (h * 2 + 0) * V:(h * 2 + 1) * V],
                                    rhs=gt0[:, :NW], start=True, stop=False)
                                nc.tensor.matmul(
                                    po[:, :NW],
                                    lhsT=w_sb[:, (h * 2 + 1) * V:(h * 2 + 2) * V],
                                    rhs=gt1[:, :NW], start=False, stop=True)
                                src = po
                            else:
                                # 3-term bf16x2: hi*Whi + lo*Whi + hi*Wlo,
                                # accumulated in PSUM rows 0..V
                                po = popool.tile([V, 512], f32, tag="po")
                                sl = slice(off, off + NW)
                                first = True
                                for c in range(2):
                                    base = (h * 2 + c) * 2 * V
                                    whi = w_sb[:, base:base + V]
                                    wlo = w_sb[:, base + V:base + 2 * V]
                                    nc.tensor.matmul(
                                        po[:, :NW], lhsT=whi, rhs=g[:, c, sl],
                                        start=first, stop=False)
                                    first = False
                                    nc.tensor.matmul(
                                        po[:, :NW], lhsT=whi, rhs=g[:, 2 + c, sl],
                                        start=False, stop=False)
                                    nc.tensor.matmul(
                                        po[:, :NW], lhsT=wlo, rhs=g[:, c, sl],
                                        start=False, stop=(c == 1))
                                src = po
                            ob = obpool.tile([V, 512], f32, tag="ob")
                            nc.scalar.activation(
                                ob[:, :NW], src[:V, :NW],
                                mybir.ActivationFunctionType.Identity,
                                bias=bias_sb[:, h:h + 1])
                            nc.sync.dma_start(
                                out[h, :, b * BS + off: b * BS + off + NW],
                                ob[:, :NW])
                            off += NW
    nc.compile()
    return nc


def get_nc(mode=MODE):
    if mode not in _NC_CACHE:
        _NC_CACHE[mode] = build_nc(mode)
    return _NC_CACHE[mode]


def _wrap_idx(a, BS):
    """[BS] int16 -> [P, BS//16]: idx i at [i % 16, i // 16], replicated x8."""
    return np.tile(a.reshape(BS // 16, 16).T, (8, 1))


def prep_inputs(features, mask_idx, head_weights, head_bias, mode=MODE):
    """Build per-core in_maps + the unpermute info."""
    NBUCK = GEO[mode]["NBUCK"]
    BS = GEO[mode]["BS"]
    HB = KH * NBUCK
    shift = {4: 15, 8: 14}[NBUCK]
    mask = (1 << shift) - 1

    feats = np.ascontiguousarray(
        np.asarray(features, dtype=np.float32).reshape(NROWS, D))
    mask_idx = np.asarray(mask_idx, dtype=np.int32)
    W = np.asarray(head_weights, dtype=np.float32)
    hbias = np.asarray(head_bias, dtype=np.float32)

    if mode == "fp32":
        feat_in = feats
    else:
        import ml_dtypes
        hi = feats.astype(ml_dtypes.bfloat16)
        lo = (feats - hi.astype(np.float32)).astype(ml_dtypes.bfloat16)
        feat_in = np.empty((NROWS, 2 * D), dtype=ml_dtypes.bfloat16)
        feat_in[:, :D] = hi
        feat_in[:, D:] = lo

    in_maps = []
    unperm = []   # per head: (order, counts)
    for c in range(NCORES):
        idx_payload = np.zeros((HB, P, BS // 16), np.int16)
        for hidx, k in enumerate(range(c * KH, (c + 1) * KH)):
            gid = mask_idx[k]
            bidx = gid >> shift
            # sort by full index (not just bucket id): within-bucket ascending
            # order gives the SDMA engines HBM-page-local read streams
            order = np.argsort(gid, kind="stable")
            counts = np.bincount(bidx, minlength=NBUCK)
            assert counts.max() <= BS, f"bucket overflow: {counts}"
            rel = (gid & mask).astype(np.int16)
            pos = 0
            for bb in range(NBUCK):
                cnt = int(counts[bb])
                padded = np.zeros(BS, np.int16)
                padded[:cnt] = rel[order[pos:pos + cnt]]
                idx_payload[hidx * NBUCK + bb] = _wrap_idx(padded, BS)
                pos += cnt
            unperm.append((order, counts))

        Wc = W[c * KH:(c + 1) * KH]          # [KH, 256, 8]
        if mode == "fp32":
            w_in = np.ascontiguousarray(
                Wc.reshape(KH, 2, P, V).transpose(2, 0, 1, 3).reshape(P, KH * 2 * V))
        else:
            import ml_dtypes
            Whi = Wc.astype(ml_dtypes.bfloat16)
            Wlo = (Wc - Whi.astype(np.float32)).astype(ml_dtypes.bfloat16)
            st = np.stack([Whi.reshape(KH, 2, P, V), Wlo.reshape(KH, 2, P, V)],
                          axis=3)            # [KH, 2, P, 2, V]
            w_in = np.ascontiguousarray(
                st.transpose(2, 0, 1, 3, 4).reshape(P, KH * 2 * 2 * V))
        bias_in = np.ascontiguousarray(hbias[c * KH:(c + 1) * KH].T)  # [V, KH]
        in_maps.append({"feat": feat_in, "idx": idx_payload,
                        "w": w_in, "bias": bias_in})
    return in_maps, unperm


def assemble_output(results, unperm, mode=MODE):
    NBUCK = GEO[mode]["NBUCK"]
    BS = GEO[mode]["BS"]
    out_full = np.empty((NCORES * KH, S, V), np.float32)
    for c in range(NCORES):
        dev = results[c]["out"]              # [KH, V, SW]
        for h in range(KH):
            k = c * KH + h
            order, counts = unperm[k]
            cols = np.concatenate(
                [np.arange(bb * BS, bb * BS + counts[bb]) for bb in range(NBUCK)])
            out_full[k, order, :] = dev[h][:, cols].T
    return out_full


def kernel(block_type_grid=None, features=None, mask_idx=None,
           head_weights=None, head_bias=None):
    nc = get_nc(MODE)
    in_maps, unperm = prep_inputs(features, mask_idx, head_weights, head_bias, MODE)
    res = run_bass_kernel_spmd(nc, in_maps, list(range(NCORES)))
    return assemble_output(res.results, unperm, MODE)
